# revision 1
# baseline (speedup 1.0000x reference)
"""Trainium2 Bass kernel for nn_MCM_37031208026850 (v16).

Strategy (8 NeuronCores, SPMD):
  - The folded value projections run at the end of the score segment,
    filling the PE drain gap while the vector engine finishes the last
    evacuations and the co reduce chain.
  - The kk=1 score pass is split by qi: co for combos {0,1} AllGathers
    and its 8 gate rows softmax while the qi=1 scores still stream, so
    conv1 starts with zero gate-wait and the second co collective hides
    under the T1 gating + conv1 work.
  - The mean-over-HW term of the pooled gate never touches the device:
    musum[i, combo] = q_i . (sum_b ksum_b) = x_i . (Wq^T kbar) + bq . kbar
    is a cheap host matvec chain, so the device only reduces per-batch
    maxes (one wide reduce_max + reduce_sum + add replaces the per-(mi,qi)
    ksum matmuls and strip arithmetic of v1 -- measured -18us/iter).
  - k-projection slices stream to DRAM per m-tile so the AllGathers
    launch as early as possible.

  - Shard the four 4096x512x4096 score GEMMs by query row: core r owns global
    rows [512r, 512(r+1)) (= batch b=r//2, image half r%2).
  - Each core projects its q-slices (cq, tq) and k-slices (ck, tk) locally,
    AllGathers the k projections (512x512 fp32 per core -> 512x4096 full),
    then streams k in 512-column chunks through the PE with q stationary.
  - The mean-over-HW term never touches the score matrix: mean = q @ ksum
    with ksum precomputed on host. Only the max needs the full scores; the
    vector engine max-reduces each PSUM tile as it is produced.
  - Tiny AllGather of the pooled co vectors (4x512 per core); every core
    computes all 16 softmax gates and selects/broadcasts the two gate images
    it needs with host-supplied one-hot matmuls.
  - The 1x1 value convs are folded on host (W512_64 @ Wv_c), computed
    full-batch per core; gating and the three 3x3 fusion convs run
    full-image per core (pair-duplicated), host takes core 2b's output.
  - Score/projection/conv matmuls run in float32r (1 cycle/row on TRN2,
    ~1.4e-4 rel err vs 2.5e-3 for bf16).
"""
import sys
sys.path.insert(0, "/opt/trn_rl_repo")

import numpy as np

import concourse.bass as bass
import concourse.mybir as mybir
import concourse.tile as tile
from concourse import bacc
from concourse import bass_utils
from concourse.masks import make_identity

B, C, H, W = 4, 512, 32, 32
HW = H * W
SCALE = 1.0 / C ** 0.5
NCORES = 8
P = 128
KT = C // P          # 4 k-tiles over channels
S = 512              # q-rows per core
NCH = 8              # global column chunks of 512
F32 = mybir.dt.float32
F32R = mybir.dt.float32r
AX = mybir.AxisListType.X
AF = mybir.ActivationFunctionType
MUL = mybir.AluOpType.mult
ADD = mybir.AluOpType.add


# ----------------------------------------------------------------------------
# host-side preparation
# ----------------------------------------------------------------------------

def host_prep(inputs):
    """Build the 8 per-core input maps from the full problem inputs."""
    xc = np.ascontiguousarray(inputs["xc"], dtype=np.float32)
    xt = np.ascontiguousarray(inputs["xt"], dtype=np.float32)
    f = lambda k: np.ascontiguousarray(inputs[k], dtype=np.float32)
    Wq_c, bq_c = f("Wq_c"), f("bq_c")
    Wk_c, bk_c = f("Wk_c"), f("bk_c")
    Wv_c, bv_c = f("Wv_c"), f("bv_c")
    Wq_t, bq_t = f("Wq_t"), f("bq_t")
    Wk_t, bk_t = f("Wk_t"), f("bk_t")
    W64, b64 = f("W512_64"), f("b512_64")
    W1, b1 = f("W1"), f("b1")
    W2, b2 = f("W2"), f("b2")
    W3, b3 = f("W3"), f("b3")

    xcG = np.ascontiguousarray(
        xc.reshape(B, C, HW).transpose(1, 0, 2).reshape(C, B * HW))
    xtT = np.ascontiguousarray(
        xt.transpose(2, 0, 1).reshape(C, B * HW))

    # ksum[c, kk*4+b]: column-sums of the k matrices per batch, /HW (mean),
    # computed from input sums so the score matrix is never needed.
    xc_sum = xc.sum(axis=(2, 3))                      # (B, C)
    xt_sum = xt.sum(axis=1)                           # (B, C)
    ks_ck = (Wk_c @ xc_sum.T + HW * bk_c[:, None]) / HW    # (C, B)
    ks_tk = (Wk_t @ xt_sum.T + HW * bk_t[:, None]) / HW    # (C, B)
    ksums = np.concatenate([ks_ck, ks_tk], axis=1)    # (C, 8) [kk*4+b]
    # host mean term: musum[i, combo=2qi+kk] = q_i . sum_b ks_b
    kbar = [ks_ck.sum(1), ks_tk.sum(1)]
    Wqs, bqs, Xrows = [Wq_c, Wq_t], [bq_c, bq_t], [xcG, xtT]
    mus_full = np.zeros((2, 2, B * HW), np.float32)   # [qi, kk, i]
    for qi in range(2):
        for kk in range(2):
            u = (Wqs[qi].T @ kbar[kk]).astype(np.float32)
            mus_full[qi, kk] = Xrows[qi].T @ u + float(bqs[qi] @ kbar[kk])

    Wcv = W64 @ Wv_c                                  # (64, C)
    bcv = W64 @ bv_c                                  # (64,)
    wcv64 = np.ascontiguousarray(np.concatenate([Wcv, Wcv], axis=0).T)   # (C,128)
    wtv64 = np.ascontiguousarray(np.concatenate([W64, W64], axis=0).T)   # (C,128)
    bcv64 = np.concatenate([bcv, bcv]).reshape(P, 1)
    b64dup = np.concatenate([b64, b64]).reshape(P, 1)

    def kmaj(a):
        # (C, n) -> (P, KT*n) k-major layout matching SBUF tiles
        n = a.shape[1]
        return a.reshape(KT, P, n).transpose(1, 0, 2).reshape(P, KT * n)

    # blob_k: k-projection weights (needed first)
    blob_k = np.hstack([kmaj(Wk_c.T), kmaj(Wk_t.T)])                  # (P, 4096)
    # blob_q: q-projection weights
    blob_q = np.hstack([kmaj(Wq_c.T), kmaj(Wq_t.T)])                  # (P, 4096)
    w3t_full = W3.transpose(1, 2, 3, 0).reshape(P, 9 * 64)
    pad = np.zeros((P - 64, 9 * 64), np.float32)
    # blob_v: value weights + ksums + sels + conv weights
    sel_pad = np.zeros((P, 2 * P), np.float32)   # filled per-core later
    blob_v_shared = np.hstack([
        kmaj(wcv64), kmaj(wtv64), kmaj(ksums),
        W1.transpose(1, 2, 3, 0).reshape(P, 9 * 64),
        W2.transpose(1, 2, 3, 0).reshape(P, 9 * 64),
        np.vstack([w3t_full[:64], pad]),
        np.vstack([w3t_full[64:], pad]),
    ])                                                                # (P, ...)
    # blob_b (fp32): all biases column-packed
    cpad = np.zeros((P - 64, 1), np.float32)
    blob_b = np.hstack([
        bq_c.reshape(KT, P).T, bk_c.reshape(KT, P).T,
        bq_t.reshape(KT, P).T, bk_t.reshape(KT, P).T,
        bcv64, b64dup,
        np.vstack([b1.reshape(64, 1), cpad]),
        np.vstack([b2.reshape(64, 1), cpad]),
        np.vstack([b3.reshape(64, 1), cpad]),
    ]).astype(np.float32)                                             # (P, 19)
    shared = {
        "blob_k": np.ascontiguousarray(blob_k, dtype=np.float32),
        "blob_q": np.ascontiguousarray(blob_q, dtype=np.float32),
        "blob_b": np.ascontiguousarray(blob_b, dtype=np.float32),
    }

    in_maps = []
    for r in range(NCORES):
        myb = r // 2
        cols = slice(S * r, S * (r + 1))
        bcols = slice(HW * myb, HW * (myb + 1))
        sel1 = np.zeros((16, P), np.float32)   # T1 = [c_co; ct_co] combos 0,1
        sel2 = np.zeros((16, P), np.float32)   # T2 = [t_co; tc_co] combos 3,2
        for p in range(P):
            sel1[(0 if p < 64 else 1) * 4 + myb, p] = 1.0
            sel2[((3 if p < 64 else 2) - 2) * 4 + myb, p] = 1.0
        mus = np.zeros((P, KT, 4), np.float32)
        for qi in range(2):
            for kk in range(2):
                mus[:, :, 2 * qi + kk] = (
                    mus_full[qi, kk, S * r:S * (r + 1)].reshape(KT, P).T)
        m = dict(shared)
        m["musum"] = np.ascontiguousarray(mus.reshape(P, KT * 4))
        m["xcq"] = np.ascontiguousarray(xcG[:, cols])
        m["xtq"] = np.ascontiguousarray(xtT[:, cols])
        m["xcb"] = np.ascontiguousarray(xcG[:, bcols])
        m["xtb"] = np.ascontiguousarray(xtT[:, bcols])
        m["blob_v"] = np.ascontiguousarray(
            np.hstack([blob_v_shared,
                       np.vstack([sel1, np.zeros((P - 16, P), np.float32)]),
                       np.vstack([sel2, np.zeros((P - 16, P), np.float32)])]),
            dtype=np.float32)
        in_maps.append(m)
    return in_maps


# ----------------------------------------------------------------------------
# device program
# ----------------------------------------------------------------------------

def build_program(time_reps: int = 1, debug: bool = False):
    """Build + bacc-compile the SPMD Bass program.

    time_reps > 1 wraps the three compute segments in For_i loops (collectives
    stay outside) so wall-clock deltas between different reps counts measure
    pure per-iteration compute time.
    """
    import contextlib
    nc = bacc.Bacc("TRN2", target_bir_lowering=False, debug=False,
                   num_devices=NCORES)

    def din(name, shape, dtype=F32R):
        return nc.dram_tensor(name, list(shape), dtype, kind="ExternalInput")

    xcq_d = din("xcq", (C, S)); xtq_d = din("xtq", (C, S))
    xcb_d = din("xcb", (C, HW)); xtb_d = din("xtb", (C, HW))
    blob_k_d = din("blob_k", (P, 2 * KT * 512))
    blob_q_d = din("blob_q", (P, 2 * KT * 512))
    VBLOB = 512 + 512 + 32 + 576 + 576 + 576 + 576 + 128 + 128
    blob_v_d = din("blob_v", (P, VBLOB))
    blob_b_d = din("blob_b", (P, 21), F32)
    musum_d = din("musum", (P, KT * 4), F32)

    outp_d = nc.dram_tensor("outp", [64, HW], F32, kind="ExternalOutput")
    if debug:
        dbg_co_d = nc.dram_tensor("dbg_co", [4, S], F32, kind="ExternalOutput")
        dbg_gates_d = nc.dram_tensor("dbg_gates", [16, HW], F32,
                                     kind="ExternalOutput")
        dbg_cv_d = nc.dram_tensor("dbg_cv", [P, HW], F32, kind="ExternalOutput")
        dbg_cq_d = nc.dram_tensor("dbg_cq", [P, KT, S], F32,
                                  kind="ExternalOutput")

    KMAJ = "(kt p) n -> p kt n"

    with tile.TileContext(nc) as tc:
      with tc.tile_pool(name="consts", bufs=1) as cons, \
           tc.tile_pool(name="dram", bufs=1, space="DRAM") as dram:
        # ---------------- static loads (phase-ordered) ----------------
        # load order tracks first use: bias blob (tiny), then the exact
        # tensors the first k-projection needs (xcq + Wk_c), then the rest
        bblob_sb = cons.tile([P, 21], F32)
        nc.sync.dma_start(bblob_sb, blob_b_d.ap())
        _xq_ctx = tc.tile_pool(name="xq", bufs=1)
        xqp = _xq_ctx.__enter__()
        xcq_sb = xqp.tile([P, KT, S], F32R)
        xtq_sb = xqp.tile([P, KT, S], F32R)
        kblob_sb = cons.tile([P, 2, KT, 512], F32R)
        qblob_sb = cons.tile([P, 2, KT, 512], F32R)
        kb_view = blob_k_d.ap().rearrange("p (k kt n) -> p k kt n",
                                          k=2, kt=KT)
        xcq_view = xcq_d.ap().rearrange(KMAJ, p=P)
        for kt in range(KT):
            nc.sync.dma_start(xcq_sb[:, kt], xcq_view[:, kt])
            nc.sync.dma_start(kblob_sb[:, 0, kt], kb_view[:, 0, kt])
            if kt == 0:
                nc.sync.dma_start(bblob_sb, blob_b_d.ap())
        xtq_view = xtq_d.ap().rearrange(KMAJ, p=P)
        for kt in range(KT):
            nc.sync.dma_start(xtq_sb[:, kt], xtq_view[:, kt])
            nc.sync.dma_start(kblob_sb[:, 1, kt], kb_view[:, 1, kt])
        qb_view = blob_q_d.ap().rearrange("p (k kt n) -> p k kt n",
                                          k=2, kt=KT)
        for half in range(2):
            for kt in range(KT):
                nc.sync.dma_start(qblob_sb[:, half, kt],
                                  qb_view[:, half, kt])
        musum_sb = cons.tile([P, KT, 4], F32)
        nc.sync.dma_start(musum_sb, musum_d.ap().rearrange(
            "p (kt c) -> p kt c", kt=KT))
        vblob_sb = cons.tile([P, VBLOB], F32R)
        nc.sync.dma_start(vblob_sb, blob_v_d.ap())
        xcb_sb = cons.tile([P, KT, HW], F32R)
        xtb_sb = cons.tile([P, KT, HW], F32R)
        nc.sync.dma_start(xcb_sb, xcb_d.ap().rearrange(KMAJ, p=P))
        nc.sync.dma_start(xtb_sb, xtb_d.ap().rearrange(KMAJ, p=P))

        # blob views
        def vsl(lo, n):
            return vblob_sb[:, lo:lo + n]

        wcv_sb = vsl(0, 512).rearrange("p (kt n) -> p kt n", kt=KT)
        wtv_sb = vsl(512, 512).rearrange("p (kt n) -> p kt n", kt=KT)
        ksums_sb = vsl(1024, 32).rearrange("p (kt n) -> p kt n", kt=KT)
        w1t_sb = vsl(1056, 576).rearrange("p (t n) -> p t n", t=9)
        w2t_sb = vsl(1632, 576).rearrange("p (t n) -> p t n", t=9)
        w3a_sb = vblob_sb[0:64, 2208:2784].rearrange("p (t n) -> p t n", t=9)
        w3b_sb = vblob_sb[0:64, 2784:3360].rearrange("p (t n) -> p t n", t=9)
        sel1_sb = vblob_sb[0:16, 3360:3488]
        sel2_sb = vblob_sb[0:16, 3488:3616]
        conv_w = [w1t_sb, w2t_sb]
        bq_sb = [bblob_sb[:, 0:4], bblob_sb[:, 8:12]]
        bk_sb = [bblob_sb[:, 4:8], bblob_sb[:, 12:16]]
        bcv_sb = bblob_sb[:, 16:17]
        b64_sb = bblob_sb[:, 17:18]
        conv_b = [bblob_sb[0:64, 18:19], bblob_sb[0:64, 19:20]]
        cb3_sb = bblob_sb[0:64, 20:21]

        ident = cons.tile([P, P], F32)
        make_identity(nc, ident)

        # persistent intermediates
        q_sb = [cons.tile([P, KT, S], F32R, name=f"q{i}") for i in range(2)]
        kslice_sb = [cons.tile([P, KT, S], F32R, name=f"ksl{i}")
                     for i in range(2)]
        cv_sb = cons.tile([P, HW], F32)
        tv_sb = cons.tile([P, HW], F32)
        strip = cons.tile([P, KT, 4, 4, 2], F32)   # [i, mi, combo, b, h]
        maxd = cons.tile([P, KT, 4, 4], F32)       # [i, mi, combo, b]
        sumd = cons.tile([P, KT, 4], F32)
        co_sb = cons.tile([P, KT, 4], F32)         # [i, mi, combo]
        co_row = [cons.tile([2, S], F32, name=f"cor{i}") for i in range(2)]            # [combo, i]
        gates_sb = [cons.tile([8, HW], F32, name=f"g{i}") for i in range(2)]
        rmax = [cons.tile([8, 1], F32, name=f"rm{i}") for i in range(2)]
        negmax = [cons.tile([8, 1], F32, name=f"nm{i}") for i in range(2)]
        expacc = [cons.tile([8, 1], F32, name=f"ea{i}") for i in range(2)]
        rsum = [cons.tile([8, 1], F32, name=f"rs{i}") for i in range(2)]
        expg = [cons.tile([8, HW], F32, name=f"eg{i}") for i in range(2)]
        gates_n = [cons.tile([8, HW], F32R, name=f"gn{i}") for i in range(2)]
        T1 = cons.tile([P, H + 2, W + 2], F32R)
        T2 = cons.tile([P, H + 2, W + 2], F32R)
        T3a = cons.tile([64, H + 2, W + 2], F32R)
        T3b = cons.tile([64, H + 2, W + 2], F32R)
        out_sb = cons.tile([64, H, W], F32)
        nc.vector.memset(T1.bitcast(F32), 0.0)
        nc.vector.memset(T2.bitcast(F32), 0.0)
        nc.vector.memset(T3a.bitcast(F32), 0.0)
        nc.vector.memset(T3b.bitcast(F32), 0.0)

        kslice_dram = [dram.tile([C, S], F32R, name=f"ksd{i}")
                       for i in range(2)]
        ag_out = [dram.tile([NCORES * C, S], F32R, addr_space="Shared",
                            name=f"ag{i}") for i in range(2)]
        co_dram = [dram.tile([2, S], F32, name=f"cod{i}") for i in range(2)]
        co_all = [dram.tile([NCORES * 2, S], F32, addr_space="Shared",
                            name=f"coa{i}") for i in range(2)]

        rep = (lambda: tc.For_i(0, time_reps, 1)) if time_reps > 1 else None

        # ---------------- segment 1: projections + values ----------------
        with tc.tile_pool(name="pj", bufs=4, space="PSUM") as pj:
          with rep() if rep else contextlib.nullcontext():
            # k projections first so the AllGathers launch early
            evac_i = 0
            for kk in range(2):
                rhs = (xcq_sb, xtq_sb)[kk]
                for m in range(KT):
                    pq = pj.tile([P, S], F32, tag="pq", name="pq")
                    for kt in range(KT):
                        nc.tensor.matmul(pq, kblob_sb[:, kk, kt,
                                                      P * m:P * (m + 1)],
                                         rhs[:, kt], start=(kt == 0),
                                         stop=(kt == KT - 1))
                    if evac_i % 2 == 0:
                        nc.vector.tensor_scalar_add(kslice_sb[kk][:, m, :], pq,
                                                    bk_sb[kk][:, m:m + 1])
                    else:
                        nc.scalar.activation(kslice_sb[kk][:, m, :], pq,
                                             AF.Identity,
                                             bias=bk_sb[kk][:, m:m + 1])
                    evac_i += 1
                    nc.sync.dma_start(
                        kslice_dram[kk].opt().rearrange(KMAJ, p=P)[:, m],
                        kslice_sb[kk][:, m])
            # q projections
            for qi in range(2):
                rhs = (xcq_sb, xtq_sb)[qi]
                for m in range(KT):
                    pq = pj.tile([P, S], F32, tag="pq", name="pq")
                    for kt in range(KT):
                        nc.tensor.matmul(pq, qblob_sb[:, qi, kt,
                                                      P * m:P * (m + 1)],
                                         rhs[:, kt], start=(kt == 0),
                                         stop=(kt == KT - 1))
                    if evac_i % 2 == 0:
                        nc.vector.tensor_scalar_add(q_sb[qi][:, m, :], pq,
                                                    bq_sb[qi][:, m:m + 1])
                    else:
                        nc.scalar.activation(q_sb[qi][:, m, :], pq,
                                             AF.Identity,
                                             bias=bq_sb[qi][:, m:m + 1])
                    evac_i += 1

        # xcq/xtq are dead after seg1: close their pool so the kchunk
        # pool can reuse the 16KB/partition
        _xq_ctx.__exit__(None, None, None)

        # ---------------- k AllGathers ----------------
        for kk in range(2):
            nc.gpsimd.collective_compute(
                "AllGather", mybir.AluOpType.bypass,
                replica_groups=[list(range(NCORES))],
                ins=[kslice_dram[kk].opt()], outs=[ag_out[kk].opt()])

        # ---------------- segment 2: scores + co ----------------
        with tc.tile_pool(name="sc", bufs=7, space="PSUM") as sc, \
             tc.tile_pool(name="fin", bufs=1, space="PSUM") as fin, \
             tc.tile_pool(name="kch", bufs=2) as kch:
          with rep() if rep else contextlib.nullcontext():
            def score_block(kk, cp, qis, kchunk):
                for h_ in range(2):
                    for mi in range(KT):
                        for qi in qis:
                            ps = sc.tile([P, 512], F32, tag="ps", name="ps")
                            for kt in range(KT):
                                nc.tensor.matmul(
                                    ps,
                                    q_sb[qi][:, kt, P * mi:P * (mi + 1)],
                                    kchunk[:, h_, kt], start=(kt == 0),
                                    stop=(kt == KT - 1))
                            nc.vector.reduce_max(
                                strip[:, mi, 2 * qi + kk, cp, h_:h_ + 1],
                                ps, axis=AX)

            def load_chunk(kk, cp):
                kchunk = kch.tile([P, 2, KT, 512], F32R, tag="kch",
                                  name="kchunk")
                nc.sync.dma_start(
                    kchunk,
                    ag_out[kk][2 * C * cp:2 * C * (cp + 1), :].rearrange(
                        "(c kt p) n -> p c kt n", c=2, kt=KT, p=P))
                return kchunk

            def co_half(qi):
                # co for combos {2qi, 2qi+1} -> AllGather qi
                sl = slice(2 * qi, 2 * qi + 2)
                nc.vector.reduce_max(maxd[:, :, sl], strip[:, :, sl],
                                     axis=AX)
                nc.vector.reduce_sum(sumd[:, :, sl], maxd[:, :, sl], axis=AX)
                nc.vector.tensor_tensor(co_sb[:, :, sl], sumd[:, :, sl],
                                        musum_sb[:, :, sl], ADD)
                for mi in range(KT):
                    ptr = fin.tile([P, P], F32, tag="ptr", name="ptr")
                    nc.tensor.transpose(ptr[:2, :], co_sb[:, mi, sl], ident)
                    nc.vector.tensor_copy(
                        co_row[qi][:, P * mi:P * (mi + 1)], ptr[:2, :])
                nc.sync.dma_start(co_dram[qi].opt(), co_row[qi])
                if time_reps <= 1:
                    nc.gpsimd.collective_compute(
                        "AllGather", mybir.AluOpType.bypass,
                        replica_groups=[list(range(NCORES))],
                        ins=[co_dram[qi].opt()], outs=[co_all[qi].opt()])

            def gates_half(qi):
                # gates rows (cmb in {2qi,2qi+1}) x batch, softmaxed
                co_view = co_all[qi].opt().rearrange(
                    "(b h c) i -> c b h i", b=4, h=2, c=2)
                for c2 in range(2):
                    nc.sync.dma_start(
                        gates_sb[qi][4 * c2:4 * (c2 + 1), :].rearrange(
                            "p (h i) -> p h i", h=2),
                        co_view[c2])
                nc.vector.reduce_max(rmax[qi], gates_sb[qi], axis=AX)
                nc.vector.tensor_scalar_mul(negmax[qi], rmax[qi], -SCALE)
                nc.scalar.activation(expg[qi], gates_sb[qi], AF.Exp,
                                     bias=negmax[qi], scale=SCALE,
                                     accum_out=expacc[qi])
                nc.vector.reciprocal(rsum[qi], expacc[qi])
                nc.vector.tensor_scalar_mul(gates_n[qi], expg[qi], rsum[qi])

            for cp in range(4):       # kk = 0: both qi per chunk
                kchunk = load_chunk(0, cp)
                score_block(0, cp, (0, 1), kchunk)
            for qi in range(2):       # kk = 1: one qi per pass
                for cp in range(4):
                    kchunk = load_chunk(1, cp)
                    score_block(1, cp, (qi,), kchunk)
                if qi == 1:
                    # folded 64-ch value projections fill the PE drain
                    # gap while the DVE finishes the last evacuations
                    for vi, (wv, vt) in enumerate(((wcv_sb, cv_sb),
                                                   (wtv_sb, tv_sb))):
                        for nh in range(2):
                            pv = sc.tile([P, 512], F32, tag="ps",
                                         name="pv")
                            for kt in range(KT):
                                nc.tensor.matmul(
                                    pv, wv[:, kt],
                                    (xcb_sb, xtb_sb)[vi][
                                        :, kt, 512 * nh:512 * (nh + 1)],
                                    start=(kt == 0), stop=(kt == KT - 1))
                            if vi == 0:
                                nc.scalar.activation(
                                    vt[:, 512 * nh:512 * (nh + 1)], pv,
                                    AF.Identity, bias=bcv_sb)
                            else:
                                nc.scalar.copy(
                                    vt[:, 512 * nh:512 * (nh + 1)], pv)
                co_half(qi)
                if qi == 0 and time_reps <= 1:
                    # qi=0 gates complete under the qi=1 score pass
                    gates_half(0)

        if time_reps > 1:
            for qi in range(2):
                nc.gpsimd.collective_compute(
                    "AllGather", mybir.AluOpType.bypass,
                    replica_groups=[list(range(NCORES))],
                    ins=[co_dram[qi].opt()], outs=[co_all[qi].opt()])

        # ---------------- segment 3: gates + fusion convs ----------------
        with tc.tile_pool(name="g", bufs=2, space="PSUM") as g:
          with rep() if rep else contextlib.nullcontext():
            if time_reps > 1:
                gates_half(0)
            gates_half(1)
            # gate selection + gating, into padded conv inputs
            # (T1 <- gates half 0 via sel1 rows 0:8, T2 <- half 1 via sel2)
            def gate_one(sel, gn, val, T):
                for nh in range(2):
                    pbg = g.tile([P, 512], F32, tag="pbg", name="pbg")
                    nc.tensor.matmul(pbg, sel,
                                     gn[:, 512 * nh:512 * (nh + 1)],
                                     start=True, stop=True)
                    reg = T[:, 1 + 16 * nh:17 + 16 * nh, 1:33]
                    nc.vector.tensor_tensor(
                        reg, pbg.rearrange("p (y x) -> p y x", y=16),
                        val[:, 512 * nh:512 * (nh + 1)].rearrange(
                            "p (y x) -> p y x", y=16), MUL)
                    nc.vector.tensor_scalar_add(reg, reg, b64_sb)

            def conv12(srcT, wi, dstT):
                for cy in range(4):
                    pc = g.tile([64, 8, 32], F32, tag="pc", name="pc")
                    for tap in range(9):
                        dy, dx = tap // 3, tap % 3
                        nc.tensor.matmul(
                            pc, conv_w[wi][:, tap, :],
                            srcT[:, 8 * cy + dy:8 * cy + dy + 8, dx:dx + 32],
                            start=(tap == 0), stop=(tap == 8))
                    nc.scalar.activation(
                        dstT[:, 1 + 8 * cy:9 + 8 * cy, 1:33], pc, AF.Relu,
                        bias=conv_b[wi], scale=1.0)

            # T1 path first (its gates landed mid-seg2); conv1 then covers
            # any residual wait on the second co-AllGather before T2
            gate_one(sel1_sb[0:8], gates_n[0], cv_sb, T1)
            conv12(T1, 0, T3a)
            gate_one(sel2_sb[0:8], gates_n[1], tv_sb, T2)
            conv12(T2, 1, T3b)
            # conv3: contraction split into two 64-channel halves
            for cy in range(4):
                pc = g.tile([64, 8, 32], F32, tag="pc", name="pc")
                for hi, (wh, Th) in enumerate(((w3a_sb, T3a), (w3b_sb, T3b))):
                    for tap in range(9):
                        dy, dx = tap // 3, tap % 3
                        nc.tensor.matmul(
                            pc, wh[:, tap, :],
                            Th[:, 8 * cy + dy:8 * cy + dy + 8, dx:dx + 32],
                            start=(hi == 0 and tap == 0),
                            stop=(hi == 1 and tap == 8))
                nc.scalar.activation(out_sb[:, 8 * cy:8 * (cy + 1), :], pc,
                                     AF.Relu, bias=cb3_sb, scale=1.0)
                nc.sync.dma_start(
                    outp_d.ap().rearrange("o (y x) -> o y x",
                                          y=H)[:, 8 * cy:8 * (cy + 1), :],
                    out_sb[:, 8 * cy:8 * (cy + 1), :])
            if debug:
                nc.sync.dma_start(dbg_co_d.ap()[0:2], co_row[0])
                nc.sync.dma_start(dbg_co_d.ap()[2:4], co_row[1])
                nc.sync.dma_start(dbg_cv_d.ap(), cv_sb)
                nc.sync.dma_start(
                    dbg_cq_d.ap(),
                    q_sb[0].bitcast(F32))

    nc.compile()
    return nc


# ----------------------------------------------------------------------------
# entry point
# ----------------------------------------------------------------------------

_CACHE = {}


def _get_nc():
    if "nc" not in _CACHE:
        _CACHE["nc"] = build_program()
    return _CACHE["nc"]


def kernel(**inputs) -> np.ndarray:
    nc = _get_nc()
    in_maps = host_prep(inputs)
    res = bass_utils.run_bass_kernel_spmd(nc, in_maps,
                                          core_ids=list(range(NCORES)))
    out = np.empty((B, 64, H, W), np.float32)
    for b in range(B):
        out[b] = res.results[2 * b]["outp"].reshape(64, H, W)
    return out


if __name__ == "__main__":
    # smoke test with random inputs
    rng = np.random.default_rng(0)
    d = {
        "xc": rng.standard_normal((B, C, H, W), np.float32),
        "xt": rng.standard_normal((B, HW, C), np.float32),
    }
    for nm, o in (("q_c", C), ("k_c", C), ("v_c", C), ("q_t", C), ("k_t", C)):
        d[f"W{nm}"] = rng.standard_normal((o, C), np.float32) * 0.02
        d[f"b{nm}"] = np.zeros(o, np.float32)
    d["W512_64"] = rng.standard_normal((64, C), np.float32) * 0.02
    d["b512_64"] = np.zeros(64, np.float32)
    for i in (1, 2, 3):
        d[f"W{i}"] = rng.standard_normal((64, 128, 3, 3), np.float32) * 0.02
        d[f"b{i}"] = np.zeros(64, np.float32)
    out = kernel(**d)
    print("out", out.shape, out.dtype, np.abs(out).max())



# revision 9
# speedup vs baseline: 1.4942x; 1.4942x over previous
"""Trainium2 Bass kernel for nn_MCM_37031208026850 (v17).

Strategy (8 NeuronCores, SPMD):
  - Folded score algebra: score_ij = q_i.k_j = qhat_i.x_j + a_i with
    qhat = (Wq^T Wk)^T x + Wk^T bq. The row-constant a_i and the exact
    mean-over-HW term are host matvecs folded into musum, so the device
    only needs max_j(qhat.x_j) per (row, batch).
  - The k side of every score GEMM is therefore the RAW input x, which the
    host replicates to all 8 cores in bf16 (resident in SBUF) - the two
    1MB-per-rank k AllGathers of v16 are gone. Only two tiny pair-wise
    co AllGathers (2x512 fp32 within each core pair) remain, since the
    softmax gate for batch b couples exactly the rows owned by cores
    2b/2b+1.
  - Scores run bf16 x bf16 (measured 913ns vs 988ns fp32r per 128x512
    tile; fp8 DoubleRow variants were measured and rejected: pure fp8 is
    1.6x faster but rel_err 3e-2 > 2e-2, error-compensated fp8 is slower
    than fp32r). PSUM accumulates fp32; the DVE max-reduces [P,1024]
    groups (one per (mi, combo, batch)) directly into maxd.
  - Shard: core r owns global score rows [512r, 512(r+1)) (batch r//2).
    Projections, values, gating and the three 3x3 convs as in v16, with
    bf16 operands everywhere on the PE.
"""
import sys
sys.path.insert(0, "/opt/trn_rl_repo")

import numpy as np
import ml_dtypes

import concourse.bass as bass
import concourse.mybir as mybir
import concourse.tile as tile
from concourse import bacc
from concourse import bass_utils
from concourse.masks import make_identity

B, C, H, W = 4, 512, 32, 32
HW = H * W
SCALE = 1.0 / C ** 0.5
NCORES = 8
P = 128
KT = C // P          # 4 k-tiles over channels
S = 512              # q-rows per core
BHW = B * HW
F32 = mybir.dt.float32
F32R = mybir.dt.float32r
BF16 = mybir.dt.bfloat16
AX = mybir.AxisListType.X
AF = mybir.ActivationFunctionType
MUL = mybir.AluOpType.mult
ADD = mybir.AluOpType.add

BF = ml_dtypes.bfloat16


def _kmaj(a):
    # (C, n) -> (P, KT, n) k-major layout matching SBUF tiles
    n = a.shape[1]
    return np.ascontiguousarray(
        a.reshape(KT, P, n).transpose(1, 0, 2))


# ----------------------------------------------------------------------------
# host-side preparation
# ----------------------------------------------------------------------------

def host_prep(inputs):
    """Build the 8 per-core input maps from the full problem inputs."""
    xc = np.ascontiguousarray(inputs["xc"], dtype=np.float32)
    xt = np.ascontiguousarray(inputs["xt"], dtype=np.float32)
    f = lambda k: np.ascontiguousarray(inputs[k], dtype=np.float32)
    Wq = [f("Wq_c"), f("Wq_t")]
    bq = [f("bq_c"), f("bq_t")]
    Wk = [f("Wk_c"), f("Wk_t")]
    bk = [f("bk_c"), f("bk_t")]
    Wv_c, bv_c = f("Wv_c"), f("bv_c")
    W64, b64 = f("W512_64"), f("b512_64")
    W1, b1 = f("W1"), f("b1")
    W2, b2 = f("W2"), f("b2")
    W3, b3 = f("W3"), f("b3")

    xcG = np.ascontiguousarray(
        xc.reshape(B, C, HW).transpose(1, 0, 2).reshape(C, BHW))
    xtT = np.ascontiguousarray(
        xt.transpose(2, 0, 1).reshape(C, BHW))
    Xs = [xcG, xtT]

    # exact mean term + row constant a_i, folded into musum:
    #   musum[i, combo] = sum_b mean_j(q_i.k_j) + 4*a_i
    #   with q = Wq x + bq, k = Wk x + bk, a_i = q_i . bk
    kbar = []
    for kk in range(2):
        ks = (Wk[kk] @ Xs[kk].reshape(C, B, HW).sum(-1)
              + HW * bk[kk][:, None]) / HW          # (C, B) per-batch k mean
        kbar.append(ks.sum(1))                      # (C,)
    mus_full = np.zeros((2, 2, BHW), np.float32)
    for qi in range(2):
        for kk in range(2):
            u = Wq[qi].T @ kbar[kk]
            mean_t = Xs[qi].T @ u + float(bq[qi] @ kbar[kk])
            a_i = Xs[qi].T @ (Wq[qi] @ bk[kk]) + float(bq[qi] @ bk[kk])
            mus_full[qi, kk] = mean_t + 4.0 * a_i

    # folded qhat projection weights: qhat = M^T x + c,  M = Wq^T Wk
    mblob = np.concatenate(
        [_kmaj((Wq[qi].T @ Wk[kk]).astype(np.float32))
         for qi in range(2) for kk in range(2)],
        axis=2).astype(BF)                          # (P, KT, 4*512)
    cbias = np.stack(
        [(Wk[kk].T @ bq[qi]).reshape(KT, P).T
         for qi in range(2) for kk in range(2)],
        axis=1)                                     # (P, 4, KT)

    # value weights (folded 64ch c-path), conv weights
    Wcv = W64 @ Wv_c
    bcv = W64 @ bv_c
    wcv64 = _kmaj(np.ascontiguousarray(np.concatenate([Wcv, Wcv], 0).T))
    wtv64 = _kmaj(np.ascontiguousarray(np.concatenate([W64, W64], 0).T))
    w3t = W3.transpose(1, 2, 3, 0).reshape(P, 9 * 64)
    pad = np.zeros((P - 64, 9 * 64), np.float32)
    vblob_shared = np.hstack([
        wcv64.reshape(P, KT * P), wtv64.reshape(P, KT * P),
        W1.transpose(1, 2, 3, 0).reshape(P, 9 * 64),
        W2.transpose(1, 2, 3, 0).reshape(P, 9 * 64),
        np.vstack([w3t[:64], pad]),
        np.vstack([w3t[64:], pad]),
    ]).astype(np.float32)                           # (P, 3328)

    cpad = np.zeros((P - 64, 1), np.float32)
    bblob = np.hstack([
        cbias.reshape(P, 16),
        np.concatenate([bcv, bcv]).reshape(P, 1),
        np.concatenate([b64, b64]).reshape(P, 1),
        np.vstack([b1.reshape(64, 1), cpad]),
        np.vstack([b2.reshape(64, 1), cpad]),
        np.vstack([b3.reshape(64, 1), cpad]),
    ]).astype(np.float32)                           # (P, 21)

    xk_c = _kmaj(xcG).astype(BF).reshape(P, KT * BHW)
    xk_t = _kmaj(xtT).astype(BF).reshape(P, KT * BHW)
    shared = {
        "mblob": np.ascontiguousarray(mblob.reshape(P, KT * 4 * 512)),
        "bblob": np.ascontiguousarray(bblob),
        "xk_c": np.ascontiguousarray(xk_c),
        "xk_t": np.ascontiguousarray(xk_t),
    }

    in_maps = []
    for r in range(NCORES):
        myb = r // 2
        cols = slice(S * r, S * (r + 1))
        bcols = slice(HW * myb, HW * (myb + 1))
        # gate-row one-hots: gates_sb rows are [c2*4 + b]; T1 = [c_co; ct_co]
        # (combos 0,1 of half 0), T2 = [t_co; tc_co] (combos 3,2 of half 1)
        sel1 = np.zeros((8, P), np.float32)
        sel2 = np.zeros((8, P), np.float32)
        for p in range(P):
            sel1[(0 if p < 64 else 1) * 4 + myb, p] = 1.0
            sel2[((3 if p < 64 else 2) - 2) * 4 + myb, p] = 1.0
        selpad = np.zeros((P - 8, P), np.float32)
        vblob = np.hstack([
            vblob_shared,
            np.vstack([sel1, selpad]),
            np.vstack([sel2, selpad]),
        ]).astype(BF)                               # (P, 3584)
        mus = np.zeros((P, KT, 4), np.float32)
        for qi in range(2):
            for kk in range(2):
                mus[:, :, 2 * qi + kk] = (
                    mus_full[qi, kk, cols].reshape(KT, P).T)
        m = dict(shared)
        m["vblob"] = np.ascontiguousarray(vblob)
        m["musum"] = np.ascontiguousarray(mus.reshape(P, KT * 4))
        m["xq_c"] = np.ascontiguousarray(
            _kmaj(xcG[:, cols]).astype(BF).reshape(P, KT * S))
        m["xq_t"] = np.ascontiguousarray(
            _kmaj(xtT[:, cols]).astype(BF).reshape(P, KT * S))
        m["xv_c"] = np.ascontiguousarray(
            _kmaj(xcG[:, bcols]).astype(BF).reshape(P, KT * HW))
        m["xv_t"] = np.ascontiguousarray(
            _kmaj(xtT[:, bcols]).astype(BF).reshape(P, KT * HW))
        in_maps.append(m)
    return in_maps


# ----------------------------------------------------------------------------
# device program
# ----------------------------------------------------------------------------

def build_program(time_reps: int = 1):
    """Build + bacc-compile the SPMD Bass program.

    time_reps > 1 wraps the three compute segments in For_i loops
    (collectives stay outside) so wall-clock deltas between different reps
    counts measure pure per-iteration compute time.
    """
    import contextlib
    nc = bacc.Bacc("TRN2", target_bir_lowering=False, debug=False,
                   num_devices=NCORES)

    def din(name, shape, dtype):
        return nc.dram_tensor(name, list(shape), dtype, kind="ExternalInput")

    xq_d = [din("xq_c", (P, KT * S), BF16), din("xq_t", (P, KT * S), BF16)]
    xk_d = [din("xk_c", (P, KT * BHW), BF16),
            din("xk_t", (P, KT * BHW), BF16)]
    xv_d = [din("xv_c", (P, KT * HW), BF16), din("xv_t", (P, KT * HW), BF16)]
    mblob_d = din("mblob", (P, KT * 4 * 512), BF16)
    VBLOB = 512 + 512 + 576 + 576 + 576 + 576 + 128 + 128
    vblob_d = din("vblob", (P, VBLOB), BF16)
    bblob_d = din("bblob", (P, 21), F32)
    musum_d = din("musum", (P, KT * 4), F32)

    outp_d = nc.dram_tensor("outp", [64, HW], F32, kind="ExternalOutput")


    with tile.TileContext(nc) as tc:
      with tc.tile_pool(name="consts", bufs=1) as cons, \
           tc.tile_pool(name="dram", bufs=1, space="DRAM") as dram:
        # ---------------- static loads (phase-ordered) ----------------
        bblob_sb = cons.tile([P, 21], F32)
        nc.sync.dma_start(bblob_sb, bblob_d.ap())
        xq_sb = [cons.tile([P, KT, S], BF16, name=f"xq{i}") for i in range(2)]
        mblob_sb = cons.tile([P, KT, 4, 512], BF16)
        mb_view = mblob_d.ap().rearrange("p (kt c n) -> p kt c n", kt=KT, c=4)
        for i in range(2):
            nc.sync.dma_start(
                xq_sb[i], xq_d[i].ap().rearrange("p (kt n) -> p kt n", kt=KT))
        for kt in range(KT):
            nc.sync.dma_start(mblob_sb[:, kt], mb_view[:, kt])
        musum_sb = cons.tile([P, KT, 4], F32)
        nc.sync.dma_start(musum_sb, musum_d.ap().rearrange(
            "p (kt c) -> p kt c", kt=KT))
        # k-side raw x, resident bf16, loaded in batch-chunks so the first
        # score groups wait only on their own chunk
        xk_sb = [cons.tile([P, KT, BHW], BF16, name=f"xk{i}")
                 for i in range(2)]
        for i in range(2):
            xk_view = xk_d[i].ap().rearrange("p (kt n) -> p kt n", kt=KT)
            for b in range(B):
                cs = slice(HW * b, HW * (b + 1))
                nc.sync.dma_start(xk_sb[i][:, :, cs], xk_view[:, :, cs])
        vblob_sb = cons.tile([P, VBLOB], BF16)
        nc.sync.dma_start(vblob_sb, vblob_d.ap())
        xv_sb = [cons.tile([P, KT, HW], BF16, name=f"xv{i}") for i in range(2)]
        for i in range(2):
            nc.sync.dma_start(
                xv_sb[i], xv_d[i].ap().rearrange("p (kt n) -> p kt n", kt=KT))

        # blob views
        def vsl(lo, n):
            return vblob_sb[:, lo:lo + n]

        wcv_sb = vsl(0, 512).rearrange("p (kt n) -> p kt n", kt=KT)
        wtv_sb = vsl(512, 512).rearrange("p (kt n) -> p kt n", kt=KT)
        w1t_sb = vsl(1024, 576).rearrange("p (t n) -> p t n", t=9)
        w2t_sb = vsl(1600, 576).rearrange("p (t n) -> p t n", t=9)
        w3a_sb = vblob_sb[0:64, 2176:2752].rearrange("p (t n) -> p t n", t=9)
        w3b_sb = vblob_sb[0:64, 2752:3328].rearrange("p (t n) -> p t n", t=9)
        sel1_sb = vblob_sb[0:8, 3328:3456]
        sel2_sb = vblob_sb[0:8, 3456:3584]
        conv_w = [w1t_sb, w2t_sb]
        cb_sb = bblob_sb[:, 0:16].rearrange("p (c kt) -> p c kt", c=4)
        bcv_sb = bblob_sb[:, 16:17]
        b64_sb = bblob_sb[:, 17:18]
        conv_b = [bblob_sb[0:64, 18:19], bblob_sb[0:64, 19:20]]
        cb3_sb = bblob_sb[0:64, 20:21]

        ident = cons.tile([P, P], F32)
        make_identity(nc, ident)

        # persistent intermediates
        q_sb = [cons.tile([P, KT, S], BF16, name=f"q{i}") for i in range(4)]
        cv_sb = cons.tile([P, HW], F32)
        tv_sb = cons.tile([P, HW], F32)
        maxd = cons.tile([P, KT, 4, 4], F32)       # [i, mi, combo, b]
        sumd = cons.tile([P, KT, 4], F32)
        co_sb = cons.tile([P, KT, 4], F32)         # [i, mi, combo]
        co_row = [cons.tile([2, S], F32, name=f"cor{i}") for i in range(2)]
        gates_sb = [cons.tile([8, HW], F32, name=f"g{i}") for i in range(2)]
        rmax = [cons.tile([8, 1], F32, name=f"rm{i}") for i in range(2)]
        negmax = [cons.tile([8, 1], F32, name=f"nm{i}") for i in range(2)]
        expacc = [cons.tile([8, 1], F32, name=f"ea{i}") for i in range(2)]
        rsum = [cons.tile([8, 1], F32, name=f"rs{i}") for i in range(2)]
        expg = [cons.tile([8, HW], F32, name=f"eg{i}") for i in range(2)]
        gates_n = [cons.tile([8, HW], BF16, name=f"gn{i}") for i in range(2)]
        T1 = cons.tile([P, H + 2, W + 2], BF16)
        T2 = cons.tile([P, H + 2, W + 2], BF16)
        T3a = cons.tile([64, H + 2, W + 2], BF16)
        T3b = cons.tile([64, H + 2, W + 2], BF16)
        out_sb = cons.tile([64, H, W], F32)
        for T in (T1, T2, T3a, T3b):
            nc.vector.memset(T.bitcast(mybir.dt.uint16), 0)

        co_dram = [dram.tile([2, S], F32, name=f"cod{i}") for i in range(2)]
        co_all = [dram.tile([NCORES * 2, S], F32, addr_space="Shared",
                            name=f"coa{i}") for i in range(2)]

        rep = (lambda: tc.For_i(0, time_reps, 1)) if time_reps > 1 else None

        # ---------------- segment 1: qhat projections ----------------
        with tc.tile_pool(name="pj", bufs=4, space="PSUM") as pj:
          with rep() if rep else contextlib.nullcontext():
            evac_i = 0
            for combo in range(4):
                qi = combo // 2
                for m in range(KT):
                    pq = pj.tile([P, S], F32, tag="pq", name="pq")
                    for kt in range(KT):
                        nc.tensor.matmul(
                            pq,
                            mblob_sb[:, kt, combo, P * m:P * (m + 1)],
                            xq_sb[qi][:, kt], start=(kt == 0),
                            stop=(kt == KT - 1))
                    if evac_i % 2 == 0:
                        nc.vector.tensor_scalar_add(
                            q_sb[combo][:, m, :], pq, cb_sb[:, combo, m:m + 1])
                    else:
                        nc.scalar.activation(
                            q_sb[combo][:, m, :], pq, AF.Identity,
                            bias=cb_sb[:, combo, m:m + 1])
                    evac_i += 1

        # ---------------- segment 2: scores + co ----------------
        with tc.tile_pool(name="sc", bufs=3, space="PSUM") as sc, \
             tc.tile_pool(name="vp", bufs=1, space="PSUM") as vp, \
             tc.tile_pool(name="fin", bufs=1, space="PSUM") as fin:
          with rep() if rep else contextlib.nullcontext():
            def score_group(combo, b, mi):
                kk = combo % 2
                ps = sc.tile([P, 1024], F32, tag="ps", name="ps")
                for h_ in range(2):
                    for kt in range(KT):
                        nc.tensor.matmul(
                            ps[:, 512 * h_:512 * (h_ + 1)],
                            q_sb[combo][:, kt, P * mi:P * (mi + 1)],
                            xk_sb[kk][:, kt,
                                      HW * b + 512 * h_:HW * b + 512 * (h_ + 1)],
                            start=(kt == 0), stop=(kt == KT - 1))
                nc.vector.reduce_max(
                    maxd[:, mi, combo, b:b + 1], ps, axis=AX)

            def co_half(qi):
                sl = slice(2 * qi, 2 * qi + 2)
                nc.vector.reduce_sum(sumd[:, :, sl], maxd[:, :, sl], axis=AX)
                nc.vector.tensor_tensor(co_sb[:, :, sl], sumd[:, :, sl],
                                        musum_sb[:, :, sl], ADD)
                for mi in range(KT):
                    ptr = fin.tile([P, P], F32, tag="ptr", name="ptr")
                    nc.tensor.transpose(ptr[:2, :], co_sb[:, mi, sl], ident)
                    nc.vector.tensor_copy(
                        co_row[qi][:, P * mi:P * (mi + 1)], ptr[:2, :])
                nc.sync.dma_start(co_dram[qi].opt(), co_row[qi])
                if time_reps <= 1:
                    nc.gpsimd.collective_compute(
                        "AllGather", mybir.AluOpType.bypass,
                        replica_groups=[list(range(NCORES))],
                        ins=[co_dram[qi].opt()], outs=[co_all[qi].opt()])

            def gates_half(qi):
                # gates rows (cmb in half qi) x batch, softmaxed
                co_view = co_all[qi].opt().rearrange(
                    "(b h c) i -> c b h i", b=4, h=2, c=2)
                for c2 in range(2):
                    nc.sync.dma_start(
                        gates_sb[qi][4 * c2:4 * (c2 + 1), :].rearrange(
                            "p (h i) -> p h i", h=2),
                        co_view[c2])
                nc.vector.reduce_max(rmax[qi], gates_sb[qi], axis=AX)
                nc.vector.tensor_scalar_mul(negmax[qi], rmax[qi], -SCALE)
                nc.scalar.activation(expg[qi], gates_sb[qi], AF.Exp,
                                     bias=negmax[qi], scale=SCALE,
                                     accum_out=expacc[qi])
                nc.vector.reciprocal(rsum[qi], expacc[qi])
                nc.vector.tensor_scalar_mul(gates_n[qi], expg[qi], rsum[qi])

            for qi in range(2):
                for combo in (2 * qi, 2 * qi + 1):
                    for b in range(B):
                        for mi in range(KT):
                            score_group(combo, b, mi)
                if qi == 1:
                    # folded 64-ch value projections fill the PE drain gap
                    for vi, (wv, vt) in enumerate(((wcv_sb, cv_sb),
                                                   (wtv_sb, tv_sb))):
                        for nh in range(2):
                            pv = vp.tile([P, 512], F32, tag="pv", name="pv")
                            for kt in range(KT):
                                nc.tensor.matmul(
                                    pv, wv[:, kt],
                                    xv_sb[vi][:, kt,
                                              512 * nh:512 * (nh + 1)],
                                    start=(kt == 0), stop=(kt == KT - 1))
                            if vi == 0:
                                nc.scalar.activation(
                                    vt[:, 512 * nh:512 * (nh + 1)], pv,
                                    AF.Identity, bias=bcv_sb)
                            else:
                                nc.scalar.copy(
                                    vt[:, 512 * nh:512 * (nh + 1)], pv)
                co_half(qi)
                if qi == 0 and time_reps <= 1:
                    gates_half(0)

        if time_reps > 1:
            for qi in range(2):
                nc.gpsimd.collective_compute(
                    "AllGather", mybir.AluOpType.bypass,
                    replica_groups=[list(range(NCORES))],
                    ins=[co_dram[qi].opt()], outs=[co_all[qi].opt()])

        # ---------------- segment 3: gates + fusion convs ----------------
        with tc.tile_pool(name="g", bufs=2, space="PSUM") as g:
          with rep() if rep else contextlib.nullcontext():
            if time_reps > 1:
                gates_half(0)
            gates_half(1)

            def gate_one(sel, gn, val, T):
                for nh in range(2):
                    pbg = g.tile([P, 512], F32, tag="pbg", name="pbg")
                    nc.tensor.matmul(pbg, sel,
                                     gn[:, 512 * nh:512 * (nh + 1)],
                                     start=True, stop=True)
                    reg = T[:, 1 + 16 * nh:17 + 16 * nh, 1:33]
                    nc.vector.tensor_tensor(
                        reg, pbg.rearrange("p (y x) -> p y x", y=16),
                        val[:, 512 * nh:512 * (nh + 1)].rearrange(
                            "p (y x) -> p y x", y=16), MUL)
                    nc.vector.tensor_scalar_add(reg, reg, b64_sb)

            def conv12(srcT, wi, dstT):
                for cy in range(4):
                    pc = g.tile([64, 8, 32], F32, tag="pc", name="pc")
                    for tap in range(9):
                        dy, dx = tap // 3, tap % 3
                        nc.tensor.matmul(
                            pc, conv_w[wi][:, tap, :],
                            srcT[:, 8 * cy + dy:8 * cy + dy + 8, dx:dx + 32],
                            start=(tap == 0), stop=(tap == 8))
                    nc.scalar.activation(
                        dstT[:, 1 + 8 * cy:9 + 8 * cy, 1:33], pc, AF.Relu,
                        bias=conv_b[wi], scale=1.0)

            gate_one(sel1_sb, gates_n[0], cv_sb, T1)
            conv12(T1, 0, T3a)
            gate_one(sel2_sb, gates_n[1], tv_sb, T2)
            conv12(T2, 1, T3b)
            for cy in range(4):
                pc = g.tile([64, 8, 32], F32, tag="pc", name="pc")
                for hi, (wh, Th) in enumerate(((w3a_sb, T3a), (w3b_sb, T3b))):
                    for tap in range(9):
                        dy, dx = tap // 3, tap % 3
                        nc.tensor.matmul(
                            pc, wh[:, tap, :],
                            Th[:, 8 * cy + dy:8 * cy + dy + 8, dx:dx + 32],
                            start=(hi == 0 and tap == 0),
                            stop=(hi == 1 and tap == 8))
                nc.scalar.activation(out_sb[:, 8 * cy:8 * (cy + 1), :], pc,
                                     AF.Relu, bias=cb3_sb, scale=1.0)
                nc.sync.dma_start(
                    outp_d.ap().rearrange("o (y x) -> o y x",
                                          y=H)[:, 8 * cy:8 * (cy + 1), :],
                    out_sb[:, 8 * cy:8 * (cy + 1), :])

    nc.compile()
    return nc


# ----------------------------------------------------------------------------
# entry point
# ----------------------------------------------------------------------------

_CACHE = {}


def _get_nc():
    if "nc" not in _CACHE:
        _CACHE["nc"] = build_program()
    return _CACHE["nc"]


def kernel(**inputs) -> np.ndarray:
    nc = _get_nc()
    in_maps = host_prep(inputs)
    res = bass_utils.run_bass_kernel_spmd(nc, in_maps,
                                          core_ids=list(range(NCORES)))
    out = np.empty((B, 64, H, W), np.float32)
    for b in range(B):
        out[b] = res.results[2 * b]["outp"].reshape(64, H, W)
    return out


if __name__ == "__main__":
    rng = np.random.default_rng(0)
    d = {
        "xc": rng.standard_normal((B, C, H, W), np.float32),
        "xt": rng.standard_normal((B, HW, C), np.float32),
    }
    for nm, o in (("q_c", C), ("k_c", C), ("v_c", C), ("q_t", C), ("k_t", C)):
        d[f"W{nm}"] = rng.standard_normal((o, C), np.float32) * 0.02
        d[f"b{nm}"] = np.zeros(o, np.float32)
    d["W512_64"] = rng.standard_normal((64, C), np.float32) * 0.02
    d["b512_64"] = np.zeros(64, np.float32)
    for i in (1, 2, 3):
        d[f"W{i}"] = rng.standard_normal((64, 128, 3, 3), np.float32) * 0.02
        d[f"b{i}"] = np.zeros(64, np.float32)
    out = kernel(**d)
    print("out", out.shape, out.dtype, np.abs(out).max())


# revision 10
# speedup vs baseline: 1.5003x; 1.0041x over previous
"""Trainium2 Bass kernel for nn_MCM_37031208026850 (v17).

Strategy (8 NeuronCores, SPMD):
  - Folded score algebra: score_ij = q_i.k_j = qhat_i.x_j + a_i with
    qhat = (Wq^T Wk)^T x + Wk^T bq. The row-constant a_i and the exact
    mean-over-HW term are host matvecs folded into musum, so the device
    only needs max_j(qhat.x_j) per (row, batch).
  - The k side of every score GEMM is therefore the RAW input x, which the
    host replicates to all 8 cores in bf16 (resident in SBUF) - the two
    1MB-per-rank k AllGathers of v16 are gone. Only two tiny pair-wise
    co AllGathers (2x512 fp32 within each core pair) remain, since the
    softmax gate for batch b couples exactly the rows owned by cores
    2b/2b+1.
  - Scores run bf16 x bf16 (measured 913ns vs 988ns fp32r per 128x512
    tile; fp8 DoubleRow variants were measured and rejected: pure fp8 is
    1.6x faster but rel_err 3e-2 > 2e-2, error-compensated fp8 is slower
    than fp32r). PSUM accumulates fp32; the DVE max-reduces [P,1024]
    groups (one per (mi, combo, batch)) directly into maxd.
  - Shard: core r owns global score rows [512r, 512(r+1)) (batch r//2).
    Projections, values, gating and the three 3x3 convs as in v16, with
    bf16 operands everywhere on the PE.
"""
import sys
sys.path.insert(0, "/opt/trn_rl_repo")

import numpy as np
import ml_dtypes

import concourse.bass as bass
import concourse.mybir as mybir
import concourse.tile as tile
from concourse import bacc
from concourse import bass_utils
from concourse.masks import make_identity

B, C, H, W = 4, 512, 32, 32
HW = H * W
SCALE = 1.0 / C ** 0.5
NCORES = 8
P = 128
KT = C // P          # 4 k-tiles over channels
S = 512              # q-rows per core
BHW = B * HW
F32 = mybir.dt.float32
F32R = mybir.dt.float32r
BF16 = mybir.dt.bfloat16
AX = mybir.AxisListType.X
AF = mybir.ActivationFunctionType
MUL = mybir.AluOpType.mult
ADD = mybir.AluOpType.add

BF = ml_dtypes.bfloat16


def _kmaj(a):
    # (C, n) -> (P, KT, n) k-major layout matching SBUF tiles
    n = a.shape[1]
    return np.ascontiguousarray(
        a.reshape(KT, P, n).transpose(1, 0, 2))


# ----------------------------------------------------------------------------
# host-side preparation
# ----------------------------------------------------------------------------

def host_prep(inputs):
    """Build the 8 per-core input maps from the full problem inputs."""
    xc = np.ascontiguousarray(inputs["xc"], dtype=np.float32)
    xt = np.ascontiguousarray(inputs["xt"], dtype=np.float32)
    f = lambda k: np.ascontiguousarray(inputs[k], dtype=np.float32)
    Wq = [f("Wq_c"), f("Wq_t")]
    bq = [f("bq_c"), f("bq_t")]
    Wk = [f("Wk_c"), f("Wk_t")]
    bk = [f("bk_c"), f("bk_t")]
    Wv_c, bv_c = f("Wv_c"), f("bv_c")
    W64, b64 = f("W512_64"), f("b512_64")
    W1, b1 = f("W1"), f("b1")
    W2, b2 = f("W2"), f("b2")
    W3, b3 = f("W3"), f("b3")

    xcG = np.ascontiguousarray(
        xc.reshape(B, C, HW).transpose(1, 0, 2).reshape(C, BHW))
    xtT = np.ascontiguousarray(
        xt.transpose(2, 0, 1).reshape(C, BHW))
    Xs = [xcG, xtT]

    # exact mean term + row constant a_i, folded into musum:
    #   musum[i, combo] = sum_b mean_j(q_i.k_j) + 4*a_i
    #   with q = Wq x + bq, k = Wk x + bk, a_i = q_i . bk
    kbar = []
    for kk in range(2):
        ks = (Wk[kk] @ Xs[kk].reshape(C, B, HW).sum(-1)
              + HW * bk[kk][:, None]) / HW          # (C, B) per-batch k mean
        kbar.append(ks.sum(1))                      # (C,)
    mus_full = np.zeros((2, 2, BHW), np.float32)
    for qi in range(2):
        for kk in range(2):
            u = Wq[qi].T @ kbar[kk]
            mean_t = Xs[qi].T @ u + float(bq[qi] @ kbar[kk])
            a_i = Xs[qi].T @ (Wq[qi] @ bk[kk]) + float(bq[qi] @ bk[kk])
            mus_full[qi, kk] = mean_t + 4.0 * a_i

    # folded qhat projection weights: qhat = M^T x + c,  M = Wq^T Wk
    mblob = np.concatenate(
        [_kmaj((Wq[qi].T @ Wk[kk]).astype(np.float32))
         for qi in range(2) for kk in range(2)],
        axis=2).astype(BF)                          # (P, KT, 4*512)
    cbias = np.stack(
        [(Wk[kk].T @ bq[qi]).reshape(KT, P).T
         for qi in range(2) for kk in range(2)],
        axis=1)                                     # (P, 4, KT)

    # value weights (folded 64ch c-path), conv weights
    Wcv = W64 @ Wv_c
    bcv = W64 @ bv_c
    wcv64 = _kmaj(np.ascontiguousarray(np.concatenate([Wcv, Wcv], 0).T))
    wtv64 = _kmaj(np.ascontiguousarray(np.concatenate([W64, W64], 0).T))
    w3t = W3.transpose(1, 2, 3, 0).reshape(P, 9 * 64)
    pad = np.zeros((P - 64, 9 * 64), np.float32)
    vblob_shared = np.hstack([
        wcv64.reshape(P, KT * P), wtv64.reshape(P, KT * P),
        W1.transpose(1, 2, 3, 0).reshape(P, 9 * 64),
        W2.transpose(1, 2, 3, 0).reshape(P, 9 * 64),
        np.vstack([w3t[:64], pad]),
        np.vstack([w3t[64:], pad]),
    ]).astype(np.float32)                           # (P, 3328)

    cpad = np.zeros((P - 64, 1), np.float32)
    bblob = np.hstack([
        cbias.reshape(P, 16),
        np.concatenate([bcv, bcv]).reshape(P, 1),
        np.concatenate([b64, b64]).reshape(P, 1),
        np.vstack([b1.reshape(64, 1), cpad]),
        np.vstack([b2.reshape(64, 1), cpad]),
        np.vstack([b3.reshape(64, 1), cpad]),
    ]).astype(np.float32)                           # (P, 21)

    xk_c = _kmaj(xcG).astype(BF).reshape(P, KT * BHW)
    xk_t = _kmaj(xtT).astype(BF).reshape(P, KT * BHW)
    shared = {
        "mblob": np.ascontiguousarray(mblob.reshape(P, KT * 4 * 512)),
        "bblob": np.ascontiguousarray(bblob),
        "xk_c": np.ascontiguousarray(xk_c),
        "xk_t": np.ascontiguousarray(xk_t),
    }

    in_maps = []
    for r in range(NCORES):
        myb = r // 2
        cols = slice(S * r, S * (r + 1))
        bcols = slice(HW * myb, HW * (myb + 1))
        # gate-row one-hots: gates_sb rows are [c2*4 + b]; T1 = [c_co; ct_co]
        # (combos 0,1 of half 0), T2 = [t_co; tc_co] (combos 3,2 of half 1)
        sel1 = np.zeros((8, P), np.float32)
        sel2 = np.zeros((8, P), np.float32)
        for p in range(P):
            sel1[(0 if p < 64 else 1) * 4 + myb, p] = 1.0
            sel2[((3 if p < 64 else 2) - 2) * 4 + myb, p] = 1.0
        selpad = np.zeros((P - 8, P), np.float32)
        vblob = np.hstack([
            vblob_shared,
            np.vstack([sel1, selpad]),
            np.vstack([sel2, selpad]),
        ]).astype(BF)                               # (P, 3584)
        mus = np.zeros((P, KT, 4), np.float32)
        for qi in range(2):
            for kk in range(2):
                mus[:, :, 2 * qi + kk] = (
                    mus_full[qi, kk, cols].reshape(KT, P).T)
        m = dict(shared)
        m["vblob"] = np.ascontiguousarray(vblob)
        m["musum"] = np.ascontiguousarray(mus.reshape(P, KT * 4))
        m["xq_c"] = np.ascontiguousarray(
            _kmaj(xcG[:, cols]).astype(BF).reshape(P, KT * S))
        m["xq_t"] = np.ascontiguousarray(
            _kmaj(xtT[:, cols]).astype(BF).reshape(P, KT * S))
        m["xv_c"] = np.ascontiguousarray(
            _kmaj(xcG[:, bcols]).astype(BF).reshape(P, KT * HW))
        m["xv_t"] = np.ascontiguousarray(
            _kmaj(xtT[:, bcols]).astype(BF).reshape(P, KT * HW))
        in_maps.append(m)
    return in_maps


# ----------------------------------------------------------------------------
# device program
# ----------------------------------------------------------------------------

def build_program(time_reps: int = 1):
    """Build + bacc-compile the SPMD Bass program.

    time_reps > 1 wraps the three compute segments in For_i loops
    (collectives stay outside) so wall-clock deltas between different reps
    counts measure pure per-iteration compute time.
    """
    import contextlib
    nc = bacc.Bacc("TRN2", target_bir_lowering=False, debug=False,
                   num_devices=NCORES)

    def din(name, shape, dtype):
        return nc.dram_tensor(name, list(shape), dtype, kind="ExternalInput")

    xq_d = [din("xq_c", (P, KT * S), BF16), din("xq_t", (P, KT * S), BF16)]
    xk_d = [din("xk_c", (P, KT * BHW), BF16),
            din("xk_t", (P, KT * BHW), BF16)]
    xv_d = [din("xv_c", (P, KT * HW), BF16), din("xv_t", (P, KT * HW), BF16)]
    mblob_d = din("mblob", (P, KT * 4 * 512), BF16)
    VBLOB = 512 + 512 + 576 + 576 + 576 + 576 + 128 + 128
    vblob_d = din("vblob", (P, VBLOB), BF16)
    bblob_d = din("bblob", (P, 21), F32)
    musum_d = din("musum", (P, KT * 4), F32)

    outp_d = nc.dram_tensor("outp", [64, HW], F32, kind="ExternalOutput")


    with tile.TileContext(nc) as tc:
      with tc.tile_pool(name="consts", bufs=1) as cons, \
           tc.tile_pool(name="dram", bufs=1, space="DRAM") as dram:
        # ---------------- static loads (phase-ordered) ----------------
        bblob_sb = cons.tile([P, 21], F32)
        nc.sync.dma_start(bblob_sb, bblob_d.ap())
        xq_sb = [cons.tile([P, KT, S], BF16, name=f"xq{i}") for i in range(2)]
        mblob_sb = cons.tile([P, KT, 4, 512], BF16)
        mb_view = mblob_d.ap().rearrange("p (kt c n) -> p kt c n", kt=KT, c=4)
        for i in range(2):
            nc.sync.dma_start(
                xq_sb[i], xq_d[i].ap().rearrange("p (kt n) -> p kt n", kt=KT))
        for kt in range(KT):
            nc.sync.dma_start(mblob_sb[:, kt], mb_view[:, kt])
        musum_sb = cons.tile([P, KT, 4], F32)
        nc.sync.dma_start(musum_sb, musum_d.ap().rearrange(
            "p (kt c) -> p kt c", kt=KT))
        # k-side raw x, resident bf16, loaded in batch-chunks so the first
        # score groups wait only on their own chunk
        xk_sb = [cons.tile([P, KT, BHW], BF16, name=f"xk{i}")
                 for i in range(2)]
        for i in range(2):
            xk_view = xk_d[i].ap().rearrange("p (kt n) -> p kt n", kt=KT)
            for b in range(B):
                cs = slice(HW * b, HW * (b + 1))
                nc.sync.dma_start(xk_sb[i][:, :, cs], xk_view[:, :, cs])
        vblob_sb = cons.tile([P, VBLOB], BF16)
        nc.sync.dma_start(vblob_sb, vblob_d.ap())
        xv_sb = [cons.tile([P, KT, HW], BF16, name=f"xv{i}") for i in range(2)]
        for i in range(2):
            nc.sync.dma_start(
                xv_sb[i], xv_d[i].ap().rearrange("p (kt n) -> p kt n", kt=KT))

        # blob views
        def vsl(lo, n):
            return vblob_sb[:, lo:lo + n]

        wcv_sb = vsl(0, 512).rearrange("p (kt n) -> p kt n", kt=KT)
        wtv_sb = vsl(512, 512).rearrange("p (kt n) -> p kt n", kt=KT)
        w1t_sb = vsl(1024, 576).rearrange("p (t n) -> p t n", t=9)
        w2t_sb = vsl(1600, 576).rearrange("p (t n) -> p t n", t=9)
        w3a_sb = vblob_sb[0:64, 2176:2752].rearrange("p (t n) -> p t n", t=9)
        w3b_sb = vblob_sb[0:64, 2752:3328].rearrange("p (t n) -> p t n", t=9)
        sel1_sb = vblob_sb[0:8, 3328:3456]
        sel2_sb = vblob_sb[0:8, 3456:3584]
        conv_w = [w1t_sb, w2t_sb]
        cb_sb = bblob_sb[:, 0:16].rearrange("p (c kt) -> p c kt", c=4)
        bcv_sb = bblob_sb[:, 16:17]
        b64_sb = bblob_sb[:, 17:18]
        conv_b = [bblob_sb[0:64, 18:19], bblob_sb[0:64, 19:20]]
        cb3_sb = bblob_sb[0:64, 20:21]

        ident = cons.tile([P, P], F32)
        make_identity(nc, ident)

        # persistent intermediates
        q_sb = [cons.tile([P, KT, S], BF16, name=f"q{i}") for i in range(4)]
        cv_sb = cons.tile([P, HW], F32)
        tv_sb = cons.tile([P, HW], F32)
        maxd = cons.tile([P, KT, 4, 4], F32)       # [i, mi, combo, b]
        sumd = cons.tile([P, KT, 4], F32)
        co_sb = cons.tile([P, KT, 4], F32)         # [i, mi, combo]
        co_row = [cons.tile([2, S], F32, name=f"cor{i}") for i in range(2)]
        gates_sb = [cons.tile([8, HW], F32, name=f"g{i}") for i in range(2)]
        rmax = [cons.tile([8, 1], F32, name=f"rm{i}") for i in range(2)]
        negmax = [cons.tile([8, 1], F32, name=f"nm{i}") for i in range(2)]
        expacc = [cons.tile([8, 1], F32, name=f"ea{i}") for i in range(2)]
        rsum = [cons.tile([8, 1], F32, name=f"rs{i}") for i in range(2)]
        expg = [cons.tile([8, HW], F32, name=f"eg{i}") for i in range(2)]
        gates_n = [cons.tile([8, HW], BF16, name=f"gn{i}") for i in range(2)]
        T1 = cons.tile([P, H + 2, W + 2], BF16)
        T2 = cons.tile([P, H + 2, W + 2], BF16)
        T3a = cons.tile([64, H + 2, W + 2], BF16)
        T3b = cons.tile([64, H + 2, W + 2], BF16)
        out_sb = cons.tile([64, H, W], F32)
        for T in (T1, T2, T3a, T3b):
            nc.vector.memset(T.bitcast(mybir.dt.uint16), 0)

        co_dram = [dram.tile([2, S], F32, name=f"cod{i}") for i in range(2)]
        co_all = [dram.tile([NCORES * 2, S], F32, addr_space="Shared",
                            name=f"coa{i}") for i in range(2)]

        rep = (lambda: tc.For_i(0, time_reps, 1)) if time_reps > 1 else None

        # ------------- segment 1+2: projections, scores + co -------------
        with tc.tile_pool(name="pj", bufs=2, space="PSUM") as pj, \
             tc.tile_pool(name="sc", bufs=2, space="PSUM") as sc, \
             tc.tile_pool(name="vp", bufs=1, space="PSUM") as vp, \
             tc.tile_pool(name="fin", bufs=1, space="PSUM") as fin:
          with rep() if rep else contextlib.nullcontext():
            evac_i = 0
            for combo in range(4):
                qi = combo // 2
                for m in range(KT):
                    pq = pj.tile([P, S], F32, tag="pq", name="pq")
                    for kt in range(KT):
                        nc.tensor.matmul(
                            pq,
                            mblob_sb[:, kt, combo, P * m:P * (m + 1)],
                            xq_sb[qi][:, kt], start=(kt == 0),
                            stop=(kt == KT - 1))
                    if evac_i % 2 == 0:
                        nc.vector.tensor_scalar_add(
                            q_sb[combo][:, m, :], pq, cb_sb[:, combo, m:m + 1])
                    else:
                        nc.scalar.activation(
                            q_sb[combo][:, m, :], pq, AF.Identity,
                            bias=cb_sb[:, combo, m:m + 1])
                    evac_i += 1

            def score_group(combo, b, mi):
                kk = combo % 2
                ps = sc.tile([P, 1024], F32, tag="ps", name="ps")
                for h_ in range(2):
                    for kt in range(KT):
                        nc.tensor.matmul(
                            ps[:, 512 * h_:512 * (h_ + 1)],
                            q_sb[combo][:, kt, P * mi:P * (mi + 1)],
                            xk_sb[kk][:, kt,
                                      HW * b + 512 * h_:HW * b + 512 * (h_ + 1)],
                            start=(kt == 0), stop=(kt == KT - 1))
                nc.vector.reduce_max(
                    maxd[:, mi, combo, b:b + 1], ps, axis=AX)

            def co_half(qi):
                sl = slice(2 * qi, 2 * qi + 2)
                nc.vector.reduce_sum(sumd[:, :, sl], maxd[:, :, sl], axis=AX)
                nc.vector.tensor_tensor(co_sb[:, :, sl], sumd[:, :, sl],
                                        musum_sb[:, :, sl], ADD)
                for mi in range(KT):
                    ptr = fin.tile([P, P], F32, tag="ptr", name="ptr")
                    nc.tensor.transpose(ptr[:2, :], co_sb[:, mi, sl], ident)
                    nc.vector.tensor_copy(
                        co_row[qi][:, P * mi:P * (mi + 1)], ptr[:2, :])
                nc.sync.dma_start(co_dram[qi].opt(), co_row[qi])
                if time_reps <= 1:
                    nc.gpsimd.collective_compute(
                        "AllGather", mybir.AluOpType.bypass,
                        replica_groups=[list(range(NCORES))],
                        ins=[co_dram[qi].opt()], outs=[co_all[qi].opt()])

            def gates_half(qi):
                # gates rows (cmb in half qi) x batch, softmaxed
                co_view = co_all[qi].opt().rearrange(
                    "(b h c) i -> c b h i", b=4, h=2, c=2)
                for c2 in range(2):
                    nc.sync.dma_start(
                        gates_sb[qi][4 * c2:4 * (c2 + 1), :].rearrange(
                            "p (h i) -> p h i", h=2),
                        co_view[c2])
                nc.vector.reduce_max(rmax[qi], gates_sb[qi], axis=AX)
                nc.vector.tensor_scalar_mul(negmax[qi], rmax[qi], -SCALE)
                nc.scalar.activation(expg[qi], gates_sb[qi], AF.Exp,
                                     bias=negmax[qi], scale=SCALE,
                                     accum_out=expacc[qi])
                nc.vector.reciprocal(rsum[qi], expacc[qi])
                nc.vector.tensor_scalar_mul(gates_n[qi], expg[qi], rsum[qi])

            for qi in range(2):
                for combo in (2 * qi, 2 * qi + 1):
                    for b in range(B):
                        for mi in range(KT):
                            score_group(combo, b, mi)
                if qi == 1:
                    # folded 64-ch value projections fill the PE drain gap
                    for vi, (wv, vt) in enumerate(((wcv_sb, cv_sb),
                                                   (wtv_sb, tv_sb))):
                        for nh in range(2):
                            pv = vp.tile([P, 512], F32, tag="pv", name="pv")
                            for kt in range(KT):
                                nc.tensor.matmul(
                                    pv, wv[:, kt],
                                    xv_sb[vi][:, kt,
                                              512 * nh:512 * (nh + 1)],
                                    start=(kt == 0), stop=(kt == KT - 1))
                            if vi == 0:
                                nc.scalar.activation(
                                    vt[:, 512 * nh:512 * (nh + 1)], pv,
                                    AF.Identity, bias=bcv_sb)
                            else:
                                nc.scalar.copy(
                                    vt[:, 512 * nh:512 * (nh + 1)], pv)
                co_half(qi)
                if qi == 0 and time_reps <= 1:
                    gates_half(0)

        if time_reps > 1:
            for qi in range(2):
                nc.gpsimd.collective_compute(
                    "AllGather", mybir.AluOpType.bypass,
                    replica_groups=[list(range(NCORES))],
                    ins=[co_dram[qi].opt()], outs=[co_all[qi].opt()])

        # ---------------- segment 3: gates + fusion convs ----------------
        with tc.tile_pool(name="g", bufs=2, space="PSUM") as g:
          with rep() if rep else contextlib.nullcontext():
            if time_reps > 1:
                gates_half(0)
            gates_half(1)

            def gate_one(sel, gn, val, T):
                for nh in range(2):
                    pbg = g.tile([P, 512], F32, tag="pbg", name="pbg")
                    nc.tensor.matmul(pbg, sel,
                                     gn[:, 512 * nh:512 * (nh + 1)],
                                     start=True, stop=True)
                    reg = T[:, 1 + 16 * nh:17 + 16 * nh, 1:33]
                    nc.vector.tensor_tensor(
                        reg, pbg.rearrange("p (y x) -> p y x", y=16),
                        val[:, 512 * nh:512 * (nh + 1)].rearrange(
                            "p (y x) -> p y x", y=16), MUL)
                    nc.vector.tensor_scalar_add(reg, reg, b64_sb)

            def conv12(srcT, wi, dstT):
                for cy in range(2):
                    pc = g.tile([64, 16, 32], F32, tag="pc", name="pc")
                    for tap in range(9):
                        dy, dx = tap // 3, tap % 3
                        nc.tensor.matmul(
                            pc, conv_w[wi][:, tap, :],
                            srcT[:, 16 * cy + dy:16 * cy + dy + 16,
                                 dx:dx + 32],
                            start=(tap == 0), stop=(tap == 8))
                    nc.scalar.activation(
                        dstT[:, 1 + 16 * cy:17 + 16 * cy, 1:33], pc, AF.Relu,
                        bias=conv_b[wi], scale=1.0)

            gate_one(sel1_sb, gates_n[0], cv_sb, T1)
            conv12(T1, 0, T3a)
            gate_one(sel2_sb, gates_n[1], tv_sb, T2)
            conv12(T2, 1, T3b)
            for cy in range(2):
                pc = g.tile([64, 16, 32], F32, tag="pc", name="pc")
                for hi, (wh, Th) in enumerate(((w3a_sb, T3a), (w3b_sb, T3b))):
                    for tap in range(9):
                        dy, dx = tap // 3, tap % 3
                        nc.tensor.matmul(
                            pc, wh[:, tap, :],
                            Th[:, 16 * cy + dy:16 * cy + dy + 16,
                               dx:dx + 32],
                            start=(hi == 0 and tap == 0),
                            stop=(hi == 1 and tap == 8))
                nc.scalar.activation(out_sb[:, 16 * cy:16 * (cy + 1), :], pc,
                                     AF.Relu, bias=cb3_sb, scale=1.0)
                nc.sync.dma_start(
                    outp_d.ap().rearrange("o (y x) -> o y x",
                                          y=H)[:, 16 * cy:16 * (cy + 1), :],
                    out_sb[:, 16 * cy:16 * (cy + 1), :])

    nc.compile()
    return nc


# ----------------------------------------------------------------------------
# entry point
# ----------------------------------------------------------------------------

_CACHE = {}


def _get_nc():
    if "nc" not in _CACHE:
        _CACHE["nc"] = build_program()
    return _CACHE["nc"]


def kernel(**inputs) -> np.ndarray:
    nc = _get_nc()
    in_maps = host_prep(inputs)
    res = bass_utils.run_bass_kernel_spmd(nc, in_maps,
                                          core_ids=list(range(NCORES)))
    out = np.empty((B, 64, H, W), np.float32)
    for b in range(B):
        out[b] = res.results[2 * b]["outp"].reshape(64, H, W)
    return out


if __name__ == "__main__":
    rng = np.random.default_rng(0)
    d = {
        "xc": rng.standard_normal((B, C, H, W), np.float32),
        "xt": rng.standard_normal((B, HW, C), np.float32),
    }
    for nm, o in (("q_c", C), ("k_c", C), ("v_c", C), ("q_t", C), ("k_t", C)):
        d[f"W{nm}"] = rng.standard_normal((o, C), np.float32) * 0.02
        d[f"b{nm}"] = np.zeros(o, np.float32)
    d["W512_64"] = rng.standard_normal((64, C), np.float32) * 0.02
    d["b512_64"] = np.zeros(64, np.float32)
    for i in (1, 2, 3):
        d[f"W{i}"] = rng.standard_normal((64, 128, 3, 3), np.float32) * 0.02
        d[f"b{i}"] = np.zeros(64, np.float32)
    out = kernel(**d)
    print("out", out.shape, out.dtype, np.abs(out).max())
